# revision 7
# baseline (speedup 1.0000x reference)
"""Trainium2 Bass kernel for nn_Attention_29738353557815.

8-way tensor-parallel over heads (core c owns q-heads {2c, 2c+1}, kv-head
c//2), fp16 datapath end to end (PSUM accumulation in f32):

  - hidden^T is DMA'd once as 64 [128, 512] fp16 tiles and stays resident in
    SBUF for all projection passes.
  - phase order is chosen so each head's AllToAll hides under later compute:
      [k/v proj] [q0/g0 proj] [h0 attention] -> AllToAll#0
      [q1/g1 proj] [h1 attention]            -> AllToAll#1
      [o-proj ht 0-7 | overlaps A2A#1] [o-proj ht 8-15]
  - k/v projections run dt-major (both accumulators advance per hT tile) so
    the first matmul issues ~1.5us in; no projection chain ever waits on more
    than one in-flight hT tile.
  - rms-norm via ones-matmul column sums folded into ln/exp on ACT; rope
    tables (cos/sin * sqrt(scale), sin halves pre-swapped) host-precomputed.
  - attention in S^T layout ([key, query] tiles). Exp runs with bias=-5 so
    pt/rowsum/ot all fit fp16 (max |logit| ~5.2; the shift cancels exactly in
    the softmax normalization). Causal AND segment masks are both
    scalar_tensor_tensor ops on DVE against per-partition threshold tables
    (fp16 2x mode), keeping the Pool queue empty so collectives never block
    mask work. Invalid (s,t) tiles are skipped at build time.
  - st matmuls are emitted 3 tiles ahead of their PV/rowsum consumers so the
    PE never drains while ACT/DVE process the softmax tiles.
  - softmax denominator via ones-matmul; normalization and sigmoid gating
    fused into one multiply; AllToAll payload fp16.
"""
import sys

if "/opt/trn_rl_repo" not in sys.path:
    sys.path.insert(0, "/opt/trn_rl_repo")

import numpy as np

import concourse.bass as bass
from concourse import bacc
import concourse.mybir as mybir
import concourse.tile as tile
from concourse.bass_utils import run_bass_kernel_spmd
from concourse.masks import make_identity

F32 = mybir.dt.float32
F16 = mybir.dt.float16
AF = mybir.ActivationFunctionType
OP = mybir.AluOpType

B, T, D = 1, 2048, 2048
NH, NKV, HD = 16, 4, 128
EPS = 1e-6
SCALE = HD ** -0.5
NCORES = 8
P = 128
NJ = T // 512      # 4 t-chunks of 512
NT = T // P        # 16 s-tiles of 128
DT = D // P        # 16 contraction tiles
TSL = T // NCORES  # 256 output rows per core
EXP_BIAS = -5.0    # keeps exp(logit) in fp16 range; cancels in softmax

_program_cache: dict = {}


def _tile_flags(seg_end: np.ndarray):
    """Per (s-tile i, t-chunk j): (skip, needs_causal, needs_seg)."""
    flags = []
    for i in range(NT):
        smin, smax = P * i, P * i + P - 1
        se_lo = int(seg_end[smin])
        se_hi = int(seg_end[smax])
        row = []
        for j in range(NJ):
            t0, t1 = 512 * j, 512 * j + 511
            skip = (t1 < smin) or (t0 >= se_hi)
            causal = (not skip) and (t0 < smax)
            segm = (not skip) and (t1 >= se_lo)
            row.append((skip, causal, segm))
        flags.append(row)
    return tuple(tuple(r) for r in flags)


def _build_program(key, use_collective=True):
    flags, unit_w = key
    nc = bacc.Bacc("TRN2", target_bir_lowering=False, debug=False,
                   num_devices=NCORES)

    hT_d = nc.dram_tensor("hT", [P, DT, T], F16, kind="ExternalInput")
    # host-prepacked, column-block-major: [q0 | q1 | g0 | g1] each [P, DT, 128]
    wqg_d = nc.dram_tensor("wqg", [4, P, DT, P], F16, kind="ExternalInput")
    wkv_d = nc.dram_tensor("wkv", [2, P, DT, P], F16, kind="ExternalInput")
    wo_d = nc.dram_tensor("wo", [P, NT, 2048], F16, kind="ExternalInput")
    tblq_d = nc.dram_tensor("tblq", [2, P, T], F16, kind="ExternalInput")
    if not unit_w:
        wqk_d = nc.dram_tensor("wqk", [P, 2], F32, kind="ExternalInput")
    iota_d = nc.dram_tensor("iota", [P, 512], F16, kind="ExternalInput")
    segrel_d = nc.dram_tensor("segrel", [P, NT, NJ], F32, kind="ExternalInput")
    srel_d = nc.dram_tensor("srel", [P, NT, NJ], F32, kind="ExternalInput")
    out_d = nc.dram_tensor("out", [TSL, D], F32, kind="ExternalOutput")

    with tile.TileContext(nc) as tc:
        with (
            tc.tile_pool(name="consts", bufs=1) as consts,
            tc.tile_pool(name="perm", bufs=1) as perm,
            tc.tile_pool(name="hw", bufs=4) as hw,
            tc.tile_pool(name="tmp", bufs=5) as tmp,
            tc.tile_pool(name="ptp", bufs=5) as ptp,
            tc.tile_pool(name="ps", bufs=1, space="PSUM") as psp,
            tc.tile_pool(name="dram", bufs=1, space="DRAM") as dram,
        ):
            # ---- resident hidden^T tiles: hTt[dt][j] = hT[:, dt, 512j:+512]
            hTt = [[consts.tile([P, 512], F16, tag=f"hT_{dt}_{j}",
                                name=f"hT_{dt}_{j}") for j in range(NJ)]
                   for dt in range(DT)]
            # weight tiles, one DMA each
            wq_sb = [consts.tile([P, DT, P], F16, tag=f"wq{c}", name=f"wq{c}")
                     for c in range(4)]        # q0, q1, g0, g1 column blocks
            wkv_sb = [[consts.tile([P, 8, P], F16, tag=f"wkv{c}{g}",
                                   name=f"wkv{c}{g}") for g in range(2)]
                      for c in range(2)]       # [k|v][dt 0-7 | dt 8-15]
            tb = {}
            for nm, idx in (("c", 0), ("s", 1)):
                tb[nm] = consts.tile([P, T], F16, tag=f"tb_{nm}",
                                     name=f"tb_{nm}")
            if not unit_w:
                wqk_sb = consts.tile([P, 2], F32)
                nc.sync.dma_start(wqk_sb[:], wqk_d[:])
            iota_sb = consts.tile([P, 512], F16)
            segrel_sb = consts.tile([P, NT, NJ], F32)
            srel_sb = consts.tile([P, NT, NJ], F32)
            ones_sb = consts.tile([P, P], F16)
            nc.vector.memset(ones_sb[:], 1.0)
            ident_sb = consts.tile([P, P], F16)
            make_identity(nc, ident_sb[:])
            eps_sb = consts.tile([P, 1], F32)
            nc.vector.memset(eps_sb[:], EPS)
            eb_sb = consts.tile([P, 1], F32)
            nc.vector.memset(eb_sb[:], EXP_BIAS)

            # ---- persistent activations ----
            qTr = [perm.tile([P, T], F16, tag=f"qTr{h}", name=f"qTr{h}")
                   for h in range(2)]
            kTr = perm.tile([P, T], F16, tag="kTr")
            gT = [perm.tile([P, T], F16, tag=f"gT{h}", name=f"gT{h}")
                  for h in range(2)]
            v_sb = perm.tile([P, NT, P], F16, tag="v_sb")

            a2a_in = [dram.tile([NCORES * P, TSL], F16, name=f"a2a_in{h}")
                      for h in range(2)]
            a2a_in8 = [a.rearrange("(s r) t -> s r t", r=P) for a in a2a_in]
            a2a_out = [dram.tile([NCORES * P, TSL], F16, name=f"a2a_out{h}")
                       for h in range(2)]

            # ================= DMA schedule (SP FIFO order) =================
            # wkv dt 0-7 first so the first k/v matmul issues ~1.5us in; hT
            # chunk j0 next; tables/weights slot into later gaps so nothing
            # the PE needs next is ever behind a bulk transfer.
            def dma_hT(j, dts):
                for dt in dts:
                    nc.sync.dma_start(hTt[dt][j][:],
                                      hT_d[:, dt, 512 * j:512 * j + 512])

            nc.sync.dma_start(wkv_sb[0][0][:], wkv_d[0, :, 0:8, :])
            nc.sync.dma_start(wkv_sb[1][0][:], wkv_d[1, :, 0:8, :])
            dma_hT(0, range(0, 8))
            nc.sync.dma_start(wkv_sb[0][1][:], wkv_d[0, :, 8:16, :])
            nc.sync.dma_start(wkv_sb[1][1][:], wkv_d[1, :, 8:16, :])
            dma_hT(0, range(8, 16))
            for nm, idx in (("c", 0), ("s", 1)):
                nc.sync.dma_start(tb[nm][:], tblq_d[idx])
            dma_hT(1, range(DT))
            nc.sync.dma_start(wq_sb[0][:], wqg_d[0])
            nc.sync.dma_start(wq_sb[2][:], wqg_d[2])
            dma_hT(2, range(DT))
            nc.sync.dma_start(iota_sb[:], iota_d[:])
            nc.sync.dma_start(segrel_sb[:], segrel_d[:])
            nc.sync.dma_start(srel_sb[:], srel_d[:])
            dma_hT(3, range(DT))
            nc.sync.dma_start(wq_sb[1][:], wqg_d[1])
            nc.sync.dma_start(wq_sb[3][:], wqg_d[3])

            # ================= shared epilogues =================
            def rope_norm_epi(mm_ps, dest, tsl, widx):
                """rms-norm + rope from a [P,512] PSUM proj block to dest."""
                qpre = tmp.tile([P, 512], F16, tag="tmp")
                nc.vector.tensor_copy(qpre[:], mm_ps[:])
                q2 = tmp.tile([P, 512], F16, tag="tmp2", bufs=2)
                nc.scalar.activation(q2[:], mm_ps[:], AF.Square)
                if not unit_w:
                    qw = tmp.tile([P, 512], F16, tag="tmp")
                    nc.vector.tensor_scalar_mul(
                        qw[:], qpre[:], wqk_sb[:, widx:widx + 1])
                    qpre = qw
                ssq_ps = psp.tile([P, 512], F32, tag="aux", bufs=1)
                nc.tensor.matmul(ssq_ps[:], ones_sb[:], q2[:],
                                 start=True, stop=True)
                rsv = tmp.tile([P, 512], F16, tag="tmp")
                nc.scalar.activation(rsv[:], ssq_ps[:], AF.Ln,
                                     scale=1.0 / HD, bias=eps_sb[:, 0:1])
                nc.scalar.activation(rsv[:], rsv[:], AF.Exp, scale=-0.5)
                tcos = tmp.tile([P, 512], F16, tag="tmp")
                nc.vector.tensor_tensor(tcos[:], qpre[:], tb["c"][:, tsl],
                                        OP.mult)
                t2 = tmp.tile([P, 512], F16, tag="tmp")
                # sin halves pre-swapped host-side; only out is shifted
                nc.vector.tensor_tensor(t2[0:64, :], qpre[64:128, :],
                                        tb["s"][64:128, tsl], OP.mult)
                nc.vector.tensor_tensor(t2[64:128, :], qpre[0:64, :],
                                        tb["s"][0:64, tsl], OP.mult)
                nc.vector.tensor_tensor(t2[:], tcos[:], t2[:], OP.add)
                nc.vector.tensor_tensor(dest, t2[:], rsv[:], OP.mult)

            def gate_epi(mm_ps, h, tsl):
                eg = tmp.tile([P, 512], F16, tag="tmp")
                nc.scalar.activation(eg[:], mm_ps[:], AF.Exp, scale=-1.0)
                nc.scalar.activation(gT[h][:, tsl], eg[:], AF.Ln, bias=1.0)

            def v_epi(mm_ps, j):
                vtmp = tmp.tile([P, 512], F16, tag="tmp")
                nc.vector.tensor_copy(vtmp[:], mm_ps[:])
                for kk in range(4):
                    tt = 4 * j + kk
                    trp = psp.tile([P, P], F16, tag="aux", bufs=1)
                    nc.tensor.transpose(
                        trp[:], vtmp[:, P * kk:P * kk + P], ident_sb[:])
                    nc.vector.tensor_copy(v_sb[:, tt, :], trp[:])

            # ================= phase A: k/v projections =================
            # dt-major: both accumulators advance per hT tile so each tile is
            # consumed right after its DMA lands.
            for j in range(NJ):
                tsl = slice(512 * j, 512 * j + 512)
                kps = psp.tile([P, 512], F32, tag="mm", bufs=3,
                               name=f"kps{j}")
                vps = psp.tile([P, 512], F32, tag="mm", bufs=3,
                               name=f"vps{j}")
                for dt in range(DT):
                    nc.tensor.matmul(kps[:], wkv_sb[0][dt // 8][:, dt % 8, :],
                                     hTt[dt][j][:],
                                     start=(dt == 0), stop=(dt == DT - 1))
                    nc.tensor.matmul(vps[:], wkv_sb[1][dt // 8][:, dt % 8, :],
                                     hTt[dt][j][:],
                                     start=(dt == 0), stop=(dt == DT - 1))
                rope_norm_epi(kps, kTr[:, tsl], tsl, widx=1)
                v_epi(vps, j)

            # ================= phases per head =================
            def proj_qg(h):
                for j in range(NJ):
                    tsl = slice(512 * j, 512 * j + 512)
                    qps = psp.tile([P, 512], F32, tag="mm", bufs=3,
                                   name=f"qps{h}{j}")
                    for dt in range(DT):
                        nc.tensor.matmul(qps[:], wq_sb[h][:, dt, :],
                                         hTt[dt][j][:],
                                         start=(dt == 0), stop=(dt == DT - 1))
                    gps = psp.tile([P, 512], F32, tag="mm", bufs=3,
                                   name=f"gps{h}{j}")
                    for dt in range(DT):
                        nc.tensor.matmul(gps[:], wq_sb[2 + h][:, dt, :],
                                         hTt[dt][j][:],
                                         start=(dt == 0), stop=(dt == DT - 1))
                    rope_norm_epi(qps, qTr[h][:, tsl], tsl, widx=0)
                    gate_epi(gps, h, tsl)

            def emit_attention(h, j):
                tsl = slice(512 * j, 512 * j + 512)
                valid = [i for i in range(NT) if not flags[i][j][0]]
                last = len(valid) - 1
                ot_ps = psp.tile([P, 512], F32, tag="acc", bufs=4,
                                 name=f"ot_{h}_{j}")
                rs_ps = psp.tile([P, 512], F32, tag="acc", bufs=4,
                                 name=f"rs_{h}_{j}")
                DEPTH = 3  # st matmuls emitted ahead of their PV consumers
                pts = {}

                def front(idx):
                    i = valid[idx]
                    _, needs_c, needs_s = flags[i][j]
                    st_ps = psp.tile([P, 512], F32, tag="mm", bufs=3,
                                     name=f"st_{h}_{j}_{i}")
                    nc.tensor.matmul(st_ps[:], kTr[:, P * i:P * i + P],
                                     qTr[h][:, tsl], start=True, stop=True)
                    pt = ptp.tile([P, 512], F16, tag="pt",
                                  name=f"pt_{h}_{j}_{i}")
                    nc.scalar.activation(pt[:], st_ps[:], AF.Exp,
                                         bias=eb_sb[:, 0:1])
                    if needs_c:
                        nc.vector.scalar_tensor_tensor(
                            out=pt[:], in0=iota_sb[:],
                            scalar=srel_sb[:, i, j:j + 1], in1=pt[:],
                            op0=OP.is_ge, op1=OP.mult)
                    if needs_s:
                        nc.vector.scalar_tensor_tensor(
                            out=pt[:], in0=iota_sb[:],
                            scalar=segrel_sb[:, i, j:j + 1], in1=pt[:],
                            op0=OP.is_lt, op1=OP.mult)
                    pts[idx] = pt

                def back(idx):
                    i = valid[idx]
                    pt = pts.pop(idx)
                    nc.tensor.matmul(ot_ps[:], v_sb[:, i, :], pt[:],
                                     start=(idx == 0), stop=(idx == last))
                    nc.tensor.matmul(rs_ps[:], ones_sb[:], pt[:],
                                     start=(idx == 0), stop=(idx == last))

                for idx in range(len(valid)):
                    front(idx)
                    if idx >= DEPTH - 1:
                        back(idx - DEPTH + 1)
                for idx in range(max(0, len(valid) - DEPTH + 1), len(valid)):
                    back(idx)

                # sig(g)/rowsum = exp(-(ln(1+e^-g) + ln(rowsum)));
                # gT holds ln(1+e^-g); EXP_BIAS cancels between ot and rs
                sg = tmp.tile([P, 512], F16, tag="tmp", name=f"sg_{h}_{j}")
                nc.scalar.activation(sg[:], rs_ps[:], AF.Ln)
                nc.vector.tensor_tensor(sg[:], sg[:], gT[h][:, tsl], OP.add)
                nc.scalar.activation(sg[:], sg[:], AF.Exp, scale=-1.0)
                ot_sb = tmp.tile([P, 512], F16, tag="tmp", name=f"otsb_{h}_{j}")
                nc.vector.tensor_copy(ot_sb[:], ot_ps[:])
                atg = tmp.tile([P, 512], F16, tag="tmp2", bufs=2,
                               name=f"atg_{h}_{j}")
                nc.vector.tensor_tensor(atg[:], ot_sb[:], sg[:], OP.mult)
                for half in range(2):
                    nc.sync.dma_start(
                        a2a_in8[h][2 * j + half, :, :],
                        atg[:, 256 * half:256 * half + 256])

            def a2a(h):
                if use_collective:
                    nc.gpsimd.collective_compute(
                        "AllToAll", OP.bypass,
                        replica_groups=[list(range(NCORES))],
                        ins=[a2a_in[h][:].opt()], outs=[a2a_out[h][:].opt()])
                else:
                    nc.sync.dma_start(a2a_out[h][:], a2a_in[h][:])

            proj_qg(0)
            for j in range(NJ):
                emit_attention(0, j)
            a2a(0)
            # wo prefetch sits in the SP queue after h0's staging DMAs and
            # before h1's; it drains during the q1/g1 + h1 attention window
            wo_sb = []
            for ht in range(NT):
                w_ = hw.tile([P, 2048], F16, tag="wo", bufs=10, name=f"wo_{ht}")
                nc.sync.dma_start(w_[:], wo_d[:, ht, :])
                wo_sb.append(w_)
            proj_qg(1)
            for j in range(NJ):
                emit_attention(1, j)
            a2a(1)

            # ================= o-proj =================
            # ht-major: 8 PSUM banks accumulate [m 0/1] x [Dc 0..3]; first 8
            # ht blocks come from a2a_out[0] and run while A2A#1 is in flight.
            # tile_wait_until defers these in the scheduling pass so o-proj
            # matmuls (which wait on collectives) are never interleaved ahead
            # of h1's attention in the PE stream.
            with tc.tile_wait_until(1.0):
                ops_tags = ["mm", "mm", "mm", "aux", "acc", "acc", "acc", "acc"]
                ops_bufs = {"mm": 3, "aux": 1, "acc": 4}
                ops = []
                for m in range(2):
                    for Dc in range(NJ):
                        tg = ops_tags[m * NJ + Dc]
                        ops.append(psp.tile([P, 512], F32, tag=tg,
                                            bufs=ops_bufs[tg],
                                            name=f"ops{m}_{Dc}"))
                ATall = []
                for hs in range(NT):
                    h, i = hs // 8, hs % 8
                    at_t = perm.tile([P, TSL], F16, tag=f"ATall{hs}",
                                     name=f"ATall{hs}")
                    nc.sync.dma_start(at_t[:], a2a_out[h][P * i:P * i + P, :])
                    ATall.append(at_t)
                for ht in range(NT):
                    at_t = ATall[ht]
                    w_full = wo_sb[ht]
                    for Dc in range(NJ):
                        for m in range(2):
                            nc.tensor.matmul(
                                ops[m * NJ + Dc][:],
                                at_t[:, P * m:P * m + P],
                                w_full[:, 512 * Dc:512 * Dc + 512],
                                start=(ht == 0), stop=(ht == NT - 1))
                # assemble [128, 1024] halves so final writes have 4 KiB runs
                for m in range(2):
                    for Dh in range(2):
                        o_sb = hw.tile([P, 1024], F32, tag="osb", bufs=2,
                                       name=f"o_{m}_{Dh}")
                        for q in range(2):
                            nc.vector.tensor_copy(
                                o_sb[:, 512 * q:512 * q + 512],
                                ops[m * NJ + 2 * Dh + q][:])
                        nc.sync.dma_start(
                            out_d[P * m:P * m + P,
                                  1024 * Dh:1024 * Dh + 1024], o_sb[:])

    nc.compile()
    _dedupe_act_table_loads(nc)
    return nc


def _dedupe_act_table_loads(nc):
    """Bacc assigns Exp->exp_and_others and Ln->natural_log, inserting a
    ~2.7us table load at every Exp<->Ln alternation. All activation funcs
    this kernel uses (Exp, Ln, Square) live in the natural_log_exp_and_others
    set, so keep one load of that set and drop the rest."""
    from concourse.hw_specs import get_activation_tables
    tabs = list(get_activation_tables(nc.m.arch).items())
    nl_exp = next(i for i, (nm, funcs) in enumerate(tabs)
                  if nm == "natural_log_exp_and_others")
    used = {ins.func for bb in nc.main_func.blocks for ins in bb.instructions
            if isinstance(ins, mybir.InstActivation)}
    assert used <= tabs[nl_exp][1], f"funcs {used} not all in natural_log_exp"
    first = True
    for bb in nc.main_func.blocks:
        keep = []
        for ins in bb.instructions:
            if isinstance(ins, mybir.InstLoadActFuncSet):
                assert ins.sync_info is None or (
                    not ins.sync_info.on_wait and not ins.sync_info.on_update)
                if first:
                    ins.act_func_set_id = nl_exp
                    keep.append(ins)
                    first = False
                continue
            keep.append(ins)
        bb.instructions[:] = keep


def _host_prep(hidden_BTD, cos_BTK, sin_BTK, segment_ids_BT, position_ids_BT,
               wq, wk, wv, wo, q_norm_w, k_norm_w):
    hidden = np.ascontiguousarray(np.asarray(hidden_BTD, dtype=np.float32)[0])
    cos = np.asarray(cos_BTK, dtype=np.float32)[0]
    sin = np.asarray(sin_BTK, dtype=np.float32)[0]
    seg = np.asarray(segment_ids_BT)[0]
    pos = np.asarray(position_ids_BT)[0]
    wq = np.asarray(wq, dtype=np.float32)
    wk = np.asarray(wk, dtype=np.float32)
    wv = np.asarray(wv, dtype=np.float32)
    wo = np.asarray(wo, dtype=np.float32)
    q_norm_w = np.asarray(q_norm_w, dtype=np.float32)
    k_norm_w = np.asarray(k_norm_w, dtype=np.float32)

    assert np.array_equal(pos, np.arange(T, dtype=pos.dtype)), \
        "kernel assumes position_ids == arange"
    assert np.all(np.diff(seg) >= 0), "kernel assumes sorted segment ids"

    # hT[p, dt, t] = hidden[t, 128*dt + p]
    hT = np.ascontiguousarray(
        hidden.T.reshape(DT, P, T).transpose(1, 0, 2).astype(np.float16))
    sqrtS = np.float32(np.sqrt(SCALE))
    signv = np.where(np.arange(HD) < HD // 2, -1.0, 1.0).astype(np.float32)
    shuf = (np.arange(HD) + HD // 2) % HD

    cosw = (cos.T * sqrtS).astype(np.float32)
    sinw = (sin.T * signv[:, None] * sqrtS).astype(np.float32)
    sinswap = sinw[shuf]  # halves swapped: see rope ops in _build_program
    tblq = np.ascontiguousarray(np.stack([cosw, sinswap]).astype(np.float16))
    unit_w = bool(np.all(q_norm_w == 1.0) and np.all(k_norm_w == 1.0))
    wqk = np.ascontiguousarray(np.stack([q_norm_w, k_norm_w], axis=1))

    # prepack wo: partition-major, block order = o-proj ht order
    # (all h0 head-blocks, then all h1)
    perm = [2 * i + h for h in range(2) for i in range(NCORES)]
    wo_p = wo.reshape(NT, P, 2048)[perm].transpose(1, 0, 2)
    wo_p = np.ascontiguousarray(wo_p.astype(np.float16))

    seg_end = np.searchsorted(seg, seg, side="right").astype(np.int64)
    iota = np.broadcast_to(np.arange(512, dtype=np.float16), (P, 512)).copy()
    segrel = np.zeros((P, NT, NJ), dtype=np.float32)
    srel = np.zeros((P, NT, NJ), dtype=np.float32)
    prange = np.arange(P, dtype=np.float32)
    for i in range(NT):
        for j in range(NJ):
            segrel[:, i, j] = seg_end[P * i:P * i + P] - 512.0 * j
            srel[:, i, j] = P * i + prange - 512.0 * j

    in_maps = []
    for c in range(NCORES):
        h0, h1 = 2 * c, 2 * c + 1
        g = c // 2
        # column blocks [q0 | q1 | g0 | g1], each repacked [P, DT, 128]
        cols = [wq[:, h0 * 256: h0 * 256 + 128],
                wq[:, h1 * 256: h1 * 256 + 128],
                wq[:, h0 * 256 + 128: h0 * 256 + 256],
                wq[:, h1 * 256 + 128: h1 * 256 + 256]]
        wqg_p = np.stack([cb.reshape(DT, P, P).transpose(1, 0, 2)
                          for cb in cols])
        wqg_p = np.ascontiguousarray(wqg_p.astype(np.float16))
        kvcols = [wk[:, g * 128:(g + 1) * 128], wv[:, g * 128:(g + 1) * 128]]
        wkv_p = np.stack([cb.reshape(DT, P, P).transpose(1, 0, 2)
                          for cb in kvcols])
        wkv_p = np.ascontiguousarray(wkv_p.astype(np.float16))
        m = {
            "hT": hT, "wqg": wqg_p, "wkv": wkv_p, "wo": wo_p,
            "tblq": tblq, "iota": iota, "segrel": segrel, "srel": srel,
        }
        if not unit_w:
            m["wqk"] = wqk
        in_maps.append(m)
    return in_maps, seg_end, unit_w


def kernel(**inputs) -> np.ndarray:
    in_maps, seg_end, unit_w = _host_prep(**inputs)
    key = (_tile_flags(seg_end), unit_w)
    if key not in _program_cache:
        _program_cache[key] = _build_program(key)
    nc = _program_cache[key]
    res = run_bass_kernel_spmd(nc, in_maps, list(range(NCORES)))
    out = np.concatenate([res.results[c]["out"] for c in range(NCORES)], axis=0)
    return out[None].astype(np.float32)


# revision 8
# speedup vs baseline: 4.8174x; 4.8174x over previous
"""Trainium2 Bass kernel for nn_Attention_29738353557815.

8-way tensor-parallel over heads (core c owns q-heads {2c, 2c+1}, kv-head
c//2), fp16 datapath end to end (PSUM accumulation in f32):

  - hidden^T is DMA'd once as 64 [128, 512] fp16 tiles and stays resident in
    SBUF for all projection passes.
  - phase order is chosen so each head's AllToAll hides under later compute:
      [k/v proj] [q0/g0 proj] [h0 attention] -> AllToAll#0
      [q1/g1 proj] [h1 attention]            -> AllToAll#1
      [o-proj ht 0-7 | overlaps A2A#1] [o-proj ht 8-15]
  - k/v projections run dt-major (both accumulators advance per hT tile) so
    the first matmul issues ~1.5us in; no projection chain ever waits on more
    than one in-flight hT tile.
  - rms-norm via ones-matmul column sums folded into ln/exp on ACT; rope
    tables (cos/sin * sqrt(scale), sin halves pre-swapped) host-precomputed.
  - attention in S^T layout ([key, query] tiles). Exp runs with bias=-5 so
    pt/rowsum/ot all fit fp16 (max |logit| ~5.2; the shift cancels exactly in
    the softmax normalization). Causal AND segment masks are both
    scalar_tensor_tensor ops on DVE against per-partition threshold tables
    (fp16 2x mode), keeping the Pool queue empty so collectives never block
    mask work. Invalid (s,t) tiles are skipped at build time.
  - st matmuls are emitted 3 tiles ahead of their PV/rowsum consumers so the
    PE never drains while ACT/DVE process the softmax tiles.
  - softmax denominator via ones-matmul; normalization and sigmoid gating
    fused into one multiply; AllToAll payload fp16.
"""
import sys

if "/opt/trn_rl_repo" not in sys.path:
    sys.path.insert(0, "/opt/trn_rl_repo")

import numpy as np

import concourse.bass as bass
from concourse import bacc
import concourse.mybir as mybir
import concourse.tile as tile
from concourse.bass_utils import run_bass_kernel_spmd
from concourse.masks import make_identity

F32 = mybir.dt.float32
F16 = mybir.dt.float16
AF = mybir.ActivationFunctionType
OP = mybir.AluOpType

B, T, D = 1, 2048, 2048
NH, NKV, HD = 16, 4, 128
EPS = 1e-6
SCALE = HD ** -0.5
NCORES = 8
P = 128
NJ = T // 512      # 4 t-chunks of 512
NT = T // P        # 16 s-tiles of 128
DT = D // P        # 16 contraction tiles
TSL = T // NCORES  # 256 output rows per core
EXP_BIAS = -5.0    # keeps exp(logit) in fp16 range; cancels in softmax

_program_cache: dict = {}


def _tile_flags(seg_end: np.ndarray):
    """Per (s-tile i, t-chunk j): (skip, needs_causal, needs_seg)."""
    flags = []
    for i in range(NT):
        smin, smax = P * i, P * i + P - 1
        se_lo = int(seg_end[smin])
        se_hi = int(seg_end[smax])
        row = []
        for j in range(NJ):
            t0, t1 = 512 * j, 512 * j + 511
            skip = (t1 < smin) or (t0 >= se_hi)
            causal = (not skip) and (t0 < smax)
            segm = (not skip) and (t1 >= se_lo)
            row.append((skip, causal, segm))
        flags.append(row)
    return tuple(tuple(r) for r in flags)


def _build_program(key, use_collective=True):
    flags, unit_w = key
    nc = bacc.Bacc("TRN2", target_bir_lowering=False, debug=False,
                   num_devices=NCORES)

    hT_d = nc.dram_tensor("hT", [P, DT, T], F16, kind="ExternalInput")
    # host-prepacked, column-block-major: [q0 | q1 | g0 | g1] each [P, DT, 128]
    wqg_d = nc.dram_tensor("wqg", [4, P, DT, P], F16, kind="ExternalInput")
    wkv_d = nc.dram_tensor("wkv", [2, P, DT, P], F16, kind="ExternalInput")
    wo_d = nc.dram_tensor("wo", [P, NT, 2048], F16, kind="ExternalInput")
    tblq_d = nc.dram_tensor("tblq", [2, P, T], F16, kind="ExternalInput")
    if not unit_w:
        wqk_d = nc.dram_tensor("wqk", [P, 2], F32, kind="ExternalInput")
    iota_d = nc.dram_tensor("iota", [P, 512], F16, kind="ExternalInput")
    segrel_d = nc.dram_tensor("segrel", [P, NT, NJ], F32, kind="ExternalInput")
    srel_d = nc.dram_tensor("srel", [P, NT, NJ], F32, kind="ExternalInput")
    out_d = nc.dram_tensor("out", [TSL, D], F32, kind="ExternalOutput")

    with tile.TileContext(nc) as tc:
        with (
            tc.tile_pool(name="consts", bufs=1) as consts,
            tc.tile_pool(name="perm", bufs=1) as perm,
            tc.tile_pool(name="hw", bufs=4) as hw,
            tc.tile_pool(name="tmp", bufs=5) as tmp,
            tc.tile_pool(name="ptp", bufs=5) as ptp,
            tc.tile_pool(name="ps", bufs=1, space="PSUM") as psp,
            tc.tile_pool(name="dram", bufs=1, space="DRAM") as dram,
        ):
            # ---- resident hidden^T tiles: hTt[dt][j] = hT[:, dt, 512j:+512]
            hTt = [[consts.tile([P, 512], F16, tag=f"hT_{dt}_{j}",
                                name=f"hT_{dt}_{j}") for j in range(NJ)]
                   for dt in range(DT)]
            # weight tiles, one DMA each
            wq_sb = [consts.tile([P, DT, P], F16, tag=f"wq{c}", name=f"wq{c}")
                     for c in range(4)]        # q0, q1, g0, g1 column blocks
            wkv_sb = [[consts.tile([P, 8, P], F16, tag=f"wkv{c}{g}",
                                   name=f"wkv{c}{g}") for g in range(2)]
                      for c in range(2)]       # [k|v][dt 0-7 | dt 8-15]
            tb = {}
            for nm, idx in (("c", 0), ("s", 1)):
                tb[nm] = consts.tile([P, T], F16, tag=f"tb_{nm}",
                                     name=f"tb_{nm}")
            if not unit_w:
                wqk_sb = consts.tile([P, 2], F32)
                nc.sync.dma_start(wqk_sb[:], wqk_d[:])
            iota_sb = consts.tile([P, 512], F16)
            segrel_sb = consts.tile([P, NT, NJ], F32)
            srel_sb = consts.tile([P, NT, NJ], F32)
            ones_sb = consts.tile([P, P], F16)
            nc.vector.memset(ones_sb[:], 1.0)
            ident_sb = consts.tile([P, P], F16)
            make_identity(nc, ident_sb[:])
            eps_sb = consts.tile([P, 1], F32)
            nc.vector.memset(eps_sb[:], EPS)
            eb_sb = consts.tile([P, 1], F32)
            nc.vector.memset(eb_sb[:], EXP_BIAS)

            # ---- persistent activations ----
            qTr = [perm.tile([P, T], F16, tag=f"qTr{h}", name=f"qTr{h}")
                   for h in range(2)]
            kTr = perm.tile([P, T], F16, tag="kTr")
            gT = [perm.tile([P, T], F16, tag=f"gT{h}", name=f"gT{h}")
                  for h in range(2)]
            v_sb = perm.tile([P, NT, P], F16, tag="v_sb")

            a2a_in = [dram.tile([NCORES * P, TSL], F16, name=f"a2a_in{h}")
                      for h in range(2)]
            a2a_in8 = [a.rearrange("(s r) t -> s r t", r=P) for a in a2a_in]
            a2a_out = [dram.tile([NCORES * P, TSL], F16, name=f"a2a_out{h}")
                       for h in range(2)]

            # ================= DMA schedule (SP FIFO order) =================
            # wkv dt 0-7 first so the first k/v matmul issues ~1.5us in; hT
            # chunk j0 next; tables/weights slot into later gaps so nothing
            # the PE needs next is ever behind a bulk transfer.
            def dma_hT(j, dts):
                for dt in dts:
                    nc.sync.dma_start(hTt[dt][j][:],
                                      hT_d[:, dt, 512 * j:512 * j + 512])

            nc.sync.dma_start(wkv_sb[0][0][:], wkv_d[0, :, 0:8, :])
            nc.sync.dma_start(wkv_sb[1][0][:], wkv_d[1, :, 0:8, :])
            dma_hT(0, range(0, 8))
            nc.sync.dma_start(wkv_sb[0][1][:], wkv_d[0, :, 8:16, :])
            nc.sync.dma_start(wkv_sb[1][1][:], wkv_d[1, :, 8:16, :])
            dma_hT(0, range(8, 16))
            for nm, idx in (("c", 0), ("s", 1)):
                nc.sync.dma_start(tb[nm][:], tblq_d[idx])
            dma_hT(1, range(DT))
            nc.sync.dma_start(wq_sb[0][:], wqg_d[0])
            nc.sync.dma_start(wq_sb[2][:], wqg_d[2])
            dma_hT(2, range(DT))
            nc.sync.dma_start(iota_sb[:], iota_d[:])
            nc.sync.dma_start(segrel_sb[:], segrel_d[:])
            nc.sync.dma_start(srel_sb[:], srel_d[:])
            dma_hT(3, range(DT))
            nc.sync.dma_start(wq_sb[1][:], wqg_d[1])
            nc.sync.dma_start(wq_sb[3][:], wqg_d[3])

            # ================= shared epilogues =================
            def rope_norm_epi(mm_ps, dest, tsl, widx):
                """rms-norm + rope from a [P,512] PSUM proj block to dest."""
                qpre = tmp.tile([P, 512], F16, tag="tmp")
                nc.vector.tensor_copy(qpre[:], mm_ps[:])
                q2 = tmp.tile([P, 512], F16, tag="tmp2", bufs=2)
                nc.scalar.activation(q2[:], mm_ps[:], AF.Square)
                if not unit_w:
                    qw = tmp.tile([P, 512], F16, tag="tmp")
                    nc.vector.tensor_scalar_mul(
                        qw[:], qpre[:], wqk_sb[:, widx:widx + 1])
                    qpre = qw
                ssq_ps = psp.tile([P, 512], F32, tag="aux", bufs=1)
                nc.tensor.matmul(ssq_ps[:], ones_sb[:], q2[:],
                                 start=True, stop=True)
                rsv = tmp.tile([P, 512], F16, tag="tmp")
                nc.scalar.activation(rsv[:], ssq_ps[:], AF.Ln,
                                     scale=1.0 / HD, bias=eps_sb[:, 0:1])
                nc.scalar.activation(rsv[:], rsv[:], AF.Exp, scale=-0.5)
                tcos = tmp.tile([P, 512], F16, tag="tmp")
                nc.vector.tensor_tensor(tcos[:], qpre[:], tb["c"][:, tsl],
                                        OP.mult)
                t2 = tmp.tile([P, 512], F16, tag="tmp")
                # sin halves pre-swapped host-side; only out is shifted
                nc.vector.tensor_tensor(t2[0:64, :], qpre[64:128, :],
                                        tb["s"][64:128, tsl], OP.mult)
                nc.vector.tensor_tensor(t2[64:128, :], qpre[0:64, :],
                                        tb["s"][0:64, tsl], OP.mult)
                nc.vector.tensor_tensor(t2[:], tcos[:], t2[:], OP.add)
                nc.vector.tensor_tensor(dest, t2[:], rsv[:], OP.mult)

            def gate_epi(mm_ps, h, tsl):
                eg = tmp.tile([P, 512], F16, tag="tmp")
                nc.scalar.activation(eg[:], mm_ps[:], AF.Exp, scale=-1.0)
                nc.scalar.activation(gT[h][:, tsl], eg[:], AF.Ln, bias=1.0)

            def v_epi(mm_ps, j):
                vtmp = tmp.tile([P, 512], F16, tag="tmp")
                nc.vector.tensor_copy(vtmp[:], mm_ps[:])
                for kk in range(4):
                    tt = 4 * j + kk
                    trp = psp.tile([P, P], F16, tag="aux", bufs=1)
                    nc.tensor.transpose(
                        trp[:], vtmp[:, P * kk:P * kk + P], ident_sb[:])
                    nc.vector.tensor_copy(v_sb[:, tt, :], trp[:])

            # ================= phase A: k/v projections =================
            # dt-major: both accumulators advance per hT tile so each tile is
            # consumed right after its DMA lands.
            for j in range(NJ):
                tsl = slice(512 * j, 512 * j + 512)
                kps = psp.tile([P, 512], F32, tag="mm", bufs=3,
                               name=f"kps{j}")
                vps = psp.tile([P, 512], F32, tag="mm", bufs=3,
                               name=f"vps{j}")
                for dt in range(DT):
                    nc.tensor.matmul(kps[:], wkv_sb[0][dt // 8][:, dt % 8, :],
                                     hTt[dt][j][:],
                                     start=(dt == 0), stop=(dt == DT - 1))
                    nc.tensor.matmul(vps[:], wkv_sb[1][dt // 8][:, dt % 8, :],
                                     hTt[dt][j][:],
                                     start=(dt == 0), stop=(dt == DT - 1))
                rope_norm_epi(kps, kTr[:, tsl], tsl, widx=1)
                v_epi(vps, j)

            # ================= phases per head =================
            def proj_qg(h):
                for j in range(NJ):
                    tsl = slice(512 * j, 512 * j + 512)
                    qps = psp.tile([P, 512], F32, tag="mm", bufs=3,
                                   name=f"qps{h}{j}")
                    for dt in range(DT):
                        nc.tensor.matmul(qps[:], wq_sb[h][:, dt, :],
                                         hTt[dt][j][:],
                                         start=(dt == 0), stop=(dt == DT - 1))
                    gps = psp.tile([P, 512], F32, tag="mm", bufs=3,
                                   name=f"gps{h}{j}")
                    for dt in range(DT):
                        nc.tensor.matmul(gps[:], wq_sb[2 + h][:, dt, :],
                                         hTt[dt][j][:],
                                         start=(dt == 0), stop=(dt == DT - 1))
                    rope_norm_epi(qps, qTr[h][:, tsl], tsl, widx=0)
                    gate_epi(gps, h, tsl)

            def emit_attention(h, j):
                tsl = slice(512 * j, 512 * j + 512)
                valid = [i for i in range(NT) if not flags[i][j][0]]
                last = len(valid) - 1
                ot_ps = psp.tile([P, 512], F32, tag="acc", bufs=4,
                                 name=f"ot_{h}_{j}")
                rs_ps = psp.tile([P, 512], F32, tag="acc", bufs=4,
                                 name=f"rs_{h}_{j}")
                DEPTH = 3  # st matmuls emitted ahead of their PV consumers
                pts = {}

                def front(idx):
                    i = valid[idx]
                    _, needs_c, needs_s = flags[i][j]
                    st_ps = psp.tile([P, 512], F32, tag="mm", bufs=3,
                                     name=f"st_{h}_{j}_{i}")
                    nc.tensor.matmul(st_ps[:], kTr[:, P * i:P * i + P],
                                     qTr[h][:, tsl], start=True, stop=True)
                    pt = ptp.tile([P, 512], F16, tag="pt",
                                  name=f"pt_{h}_{j}_{i}")
                    nc.scalar.activation(pt[:], st_ps[:], AF.Exp,
                                         bias=eb_sb[:, 0:1])
                    if needs_c:
                        nc.vector.scalar_tensor_tensor(
                            out=pt[:], in0=iota_sb[:],
                            scalar=srel_sb[:, i, j:j + 1], in1=pt[:],
                            op0=OP.is_ge, op1=OP.mult)
                    if needs_s:
                        nc.vector.scalar_tensor_tensor(
                            out=pt[:], in0=iota_sb[:],
                            scalar=segrel_sb[:, i, j:j + 1], in1=pt[:],
                            op0=OP.is_lt, op1=OP.mult)
                    pts[idx] = pt

                def back(idx):
                    i = valid[idx]
                    pt = pts.pop(idx)
                    nc.tensor.matmul(ot_ps[:], v_sb[:, i, :], pt[:],
                                     start=(idx == 0), stop=(idx == last))
                    nc.tensor.matmul(rs_ps[:], ones_sb[:], pt[:],
                                     start=(idx == 0), stop=(idx == last))

                for idx in range(len(valid)):
                    front(idx)
                    if idx >= DEPTH - 1:
                        back(idx - DEPTH + 1)
                for idx in range(max(0, len(valid) - DEPTH + 1), len(valid)):
                    back(idx)

                # sig(g)/rowsum = exp(-(ln(1+e^-g) + ln(rowsum)));
                # gT holds ln(1+e^-g); EXP_BIAS cancels between ot and rs
                sg = tmp.tile([P, 512], F16, tag="tmp", name=f"sg_{h}_{j}")
                nc.scalar.activation(sg[:], rs_ps[:], AF.Ln)
                nc.vector.tensor_tensor(sg[:], sg[:], gT[h][:, tsl], OP.add)
                nc.scalar.activation(sg[:], sg[:], AF.Exp, scale=-1.0)
                ot_sb = tmp.tile([P, 512], F16, tag="tmp", name=f"otsb_{h}_{j}")
                nc.vector.tensor_copy(ot_sb[:], ot_ps[:])
                atg = tmp.tile([P, 512], F16, tag="tmp2", bufs=2,
                               name=f"atg_{h}_{j}")
                nc.vector.tensor_tensor(atg[:], ot_sb[:], sg[:], OP.mult)
                for half in range(2):
                    nc.sync.dma_start(
                        a2a_in8[h][2 * j + half, :, :],
                        atg[:, 256 * half:256 * half + 256])

            def a2a(h):
                if use_collective:
                    nc.gpsimd.collective_compute(
                        "AllToAll", OP.bypass,
                        replica_groups=[list(range(NCORES))],
                        ins=[a2a_in[h][:].opt()], outs=[a2a_out[h][:].opt()])
                else:
                    nc.sync.dma_start(a2a_out[h][:], a2a_in[h][:])

            proj_qg(0)
            for j in range(NJ):
                emit_attention(0, j)
            a2a(0)
            # wo prefetch sits in the SP queue after h0's staging DMAs and
            # before h1's; it drains during the q1/g1 + h1 attention window
            wo_sb = []
            for ht in range(NT):
                w_ = hw.tile([P, 2048], F16, tag="wo", bufs=10, name=f"wo_{ht}")
                nc.sync.dma_start(w_[:], wo_d[:, ht, :])
                wo_sb.append(w_)
            proj_qg(1)
            for j in range(NJ):
                emit_attention(1, j)
            a2a(1)

            # ================= o-proj =================
            # ht-major: 8 PSUM banks accumulate [m 0/1] x [Dc 0..3]; first 8
            # ht blocks come from a2a_out[0] and run while A2A#1 is in flight.
            # tile_wait_until defers these in the scheduling pass so o-proj
            # matmuls (which wait on collectives) are never interleaved ahead
            # of h1's attention in the PE stream.
            with tc.tile_wait_until(0.132):
                ops_tags = ["mm", "mm", "mm", "aux", "acc", "acc", "acc", "acc"]
                ops_bufs = {"mm": 3, "aux": 1, "acc": 4}
                ops = []
                for m in range(2):
                    for Dc in range(NJ):
                        tg = ops_tags[m * NJ + Dc]
                        ops.append(psp.tile([P, 512], F32, tag=tg,
                                            bufs=ops_bufs[tg],
                                            name=f"ops{m}_{Dc}"))
                ATall = []
                for hs in range(NT):
                    h, i = hs // 8, hs % 8
                    at_t = perm.tile([P, TSL], F16, tag=f"ATall{hs}",
                                     name=f"ATall{hs}")
                    nc.sync.dma_start(at_t[:], a2a_out[h][P * i:P * i + P, :])
                    ATall.append(at_t)
                for ht in range(NT):
                    at_t = ATall[ht]
                    w_full = wo_sb[ht]
                    for Dc in range(NJ):
                        for m in range(2):
                            nc.tensor.matmul(
                                ops[m * NJ + Dc][:],
                                at_t[:, P * m:P * m + P],
                                w_full[:, 512 * Dc:512 * Dc + 512],
                                start=(ht == 0), stop=(ht == NT - 1))
                # assemble [128, 1024] halves so final writes have 4 KiB runs
                for m in range(2):
                    for Dh in range(2):
                        o_sb = hw.tile([P, 1024], F32, tag="osb", bufs=2,
                                       name=f"o_{m}_{Dh}")
                        for q in range(2):
                            nc.vector.tensor_copy(
                                o_sb[:, 512 * q:512 * q + 512],
                                ops[m * NJ + 2 * Dh + q][:])
                        nc.sync.dma_start(
                            out_d[P * m:P * m + P,
                                  1024 * Dh:1024 * Dh + 1024], o_sb[:])

    nc.compile()
    _dedupe_act_table_loads(nc)
    return nc


def _dedupe_act_table_loads(nc):
    """Bacc assigns Exp->exp_and_others and Ln->natural_log, inserting a
    ~2.7us table load at every Exp<->Ln alternation. All activation funcs
    this kernel uses (Exp, Ln, Square) live in the natural_log_exp_and_others
    set, so keep one load of that set and drop the rest."""
    from concourse.hw_specs import get_activation_tables
    tabs = list(get_activation_tables(nc.m.arch).items())
    nl_exp = next(i for i, (nm, funcs) in enumerate(tabs)
                  if nm == "natural_log_exp_and_others")
    used = {ins.func for bb in nc.main_func.blocks for ins in bb.instructions
            if isinstance(ins, mybir.InstActivation)}
    assert used <= tabs[nl_exp][1], f"funcs {used} not all in natural_log_exp"
    first = True
    for bb in nc.main_func.blocks:
        keep = []
        for ins in bb.instructions:
            if isinstance(ins, mybir.InstLoadActFuncSet):
                assert ins.sync_info is None or (
                    not ins.sync_info.on_wait and not ins.sync_info.on_update)
                if first:
                    ins.act_func_set_id = nl_exp
                    keep.append(ins)
                    first = False
                continue
            keep.append(ins)
        bb.instructions[:] = keep


def _host_prep(hidden_BTD, cos_BTK, sin_BTK, segment_ids_BT, position_ids_BT,
               wq, wk, wv, wo, q_norm_w, k_norm_w):
    hidden = np.ascontiguousarray(np.asarray(hidden_BTD, dtype=np.float32)[0])
    cos = np.asarray(cos_BTK, dtype=np.float32)[0]
    sin = np.asarray(sin_BTK, dtype=np.float32)[0]
    seg = np.asarray(segment_ids_BT)[0]
    pos = np.asarray(position_ids_BT)[0]
    wq = np.asarray(wq, dtype=np.float32)
    wk = np.asarray(wk, dtype=np.float32)
    wv = np.asarray(wv, dtype=np.float32)
    wo = np.asarray(wo, dtype=np.float32)
    q_norm_w = np.asarray(q_norm_w, dtype=np.float32)
    k_norm_w = np.asarray(k_norm_w, dtype=np.float32)

    assert np.array_equal(pos, np.arange(T, dtype=pos.dtype)), \
        "kernel assumes position_ids == arange"
    assert np.all(np.diff(seg) >= 0), "kernel assumes sorted segment ids"

    # hT[p, dt, t] = hidden[t, 128*dt + p]
    hT = np.ascontiguousarray(
        hidden.T.reshape(DT, P, T).transpose(1, 0, 2).astype(np.float16))
    sqrtS = np.float32(np.sqrt(SCALE))
    signv = np.where(np.arange(HD) < HD // 2, -1.0, 1.0).astype(np.float32)
    shuf = (np.arange(HD) + HD // 2) % HD

    cosw = (cos.T * sqrtS).astype(np.float32)
    sinw = (sin.T * signv[:, None] * sqrtS).astype(np.float32)
    sinswap = sinw[shuf]  # halves swapped: see rope ops in _build_program
    tblq = np.ascontiguousarray(np.stack([cosw, sinswap]).astype(np.float16))
    unit_w = bool(np.all(q_norm_w == 1.0) and np.all(k_norm_w == 1.0))
    wqk = np.ascontiguousarray(np.stack([q_norm_w, k_norm_w], axis=1))

    # prepack wo: partition-major, block order = o-proj ht order
    # (all h0 head-blocks, then all h1)
    perm = [2 * i + h for h in range(2) for i in range(NCORES)]
    wo_p = wo.reshape(NT, P, 2048)[perm].transpose(1, 0, 2)
    wo_p = np.ascontiguousarray(wo_p.astype(np.float16))

    seg_end = np.searchsorted(seg, seg, side="right").astype(np.int64)
    iota = np.broadcast_to(np.arange(512, dtype=np.float16), (P, 512)).copy()
    segrel = np.zeros((P, NT, NJ), dtype=np.float32)
    srel = np.zeros((P, NT, NJ), dtype=np.float32)
    prange = np.arange(P, dtype=np.float32)
    for i in range(NT):
        for j in range(NJ):
            segrel[:, i, j] = seg_end[P * i:P * i + P] - 512.0 * j
            srel[:, i, j] = P * i + prange - 512.0 * j

    in_maps = []
    for c in range(NCORES):
        h0, h1 = 2 * c, 2 * c + 1
        g = c // 2
        # column blocks [q0 | q1 | g0 | g1], each repacked [P, DT, 128]
        cols = [wq[:, h0 * 256: h0 * 256 + 128],
                wq[:, h1 * 256: h1 * 256 + 128],
                wq[:, h0 * 256 + 128: h0 * 256 + 256],
                wq[:, h1 * 256 + 128: h1 * 256 + 256]]
        wqg_p = np.stack([cb.reshape(DT, P, P).transpose(1, 0, 2)
                          for cb in cols])
        wqg_p = np.ascontiguousarray(wqg_p.astype(np.float16))
        kvcols = [wk[:, g * 128:(g + 1) * 128], wv[:, g * 128:(g + 1) * 128]]
        wkv_p = np.stack([cb.reshape(DT, P, P).transpose(1, 0, 2)
                          for cb in kvcols])
        wkv_p = np.ascontiguousarray(wkv_p.astype(np.float16))
        m = {
            "hT": hT, "wqg": wqg_p, "wkv": wkv_p, "wo": wo_p,
            "tblq": tblq, "iota": iota, "segrel": segrel, "srel": srel,
        }
        if not unit_w:
            m["wqk"] = wqk
        in_maps.append(m)
    return in_maps, seg_end, unit_w


def kernel(**inputs) -> np.ndarray:
    in_maps, seg_end, unit_w = _host_prep(**inputs)
    key = (_tile_flags(seg_end), unit_w)
    if key not in _program_cache:
        _program_cache[key] = _build_program(key)
    nc = _program_cache[key]
    res = run_bass_kernel_spmd(nc, in_maps, list(range(NCORES)))
    out = np.concatenate([res.results[c]["out"] for c in range(NCORES)], axis=0)
    return out[None].astype(np.float32)
